# revision 14
# baseline (speedup 1.0000x reference)
"""Single-head attention (B=8, T=2048, E=1024, D=128) on 8 Trainium2 NeuronCores.

Strategy (data-parallel over batch, one batch element per core), v2:
  host: pre-transpose x -> xT[b] = x[b].T (E on rows), cast to fp16.
  device, per core (flash-style, fully pipelined over T-quarters):
    - x streams in [128,1024] chunks on TWO HW DGE queues (sync + scalar)
    - per quarter h: qT/kT (fp16, pre-scaled by D**-0.25) and vT via PE
      (fp16 matmuls, full rate); V natural layout via 4 PE transposes
    - score/AV "units" are interleaved with projections as soon as their
      kT block + qT span exist (sweep A: spans 0,1; sweep B: spans 2,3 --
      halves the persistent ot PSUM banks so everything fits in 8 banks):
        unit(kb-pair, s): 2 score MMs into a 2-bank PSUM pair,
        ONE exp over [128,1024] (halves ACT instruction overhead),
        2 AV MMs accumulating OT[d, q] in a persistent span bank,
        denominator accumulated on DVE/GpSimd in fp16 SBUF (replaces the
        64 ones-matmuls of v1 -- saves ~17us of PE time)
    - span finalize: l = ones[128,1].T @ acc (M=1 matmul), 1/l via
      reciprocal_approx_fast, broadcast across partitions via K=1 matmul,
      OT * (1/l) on DVE, DMA out
    - store outT [D, T] f32; host transposes back to [T, D].
"""

import os
import sys

for _p in ("/opt/trn_rl_repo",):
    if _p not in sys.path and os.path.isdir(_p):
        sys.path.append(_p)

import numpy as np

import concourse.bass as bass
import concourse.tile as tile
from concourse import mybir
from concourse.masks import make_identity
from concourse.vector_clock import ScopedClock

B, T, E, D = 8, 2048, 1024, 128
EC = E // 128           # 8 E-chunks of 128 partitions
NSPAN = 4               # query spans of 512
SPAN = T // NSPAN       # 512
NKB = T // 128          # 16 key blocks
NPAIR = NKB // 2        # 8 key-block pairs
F32 = mybir.dt.float32
F32R = mybir.dt.float32r
BF16 = mybir.dt.bfloat16
F16 = mybir.dt.float16

_MAX_DRAIN_WAITS = 1


def _drain_and_barrier_split(self, tick_clock, wait_clock):
    # This walrus build rejects CTRL instructions carrying more than one sync
    # wait, so spread the kernel-tail drain's waits over single-wait NOPs.
    nc = self.nc
    collector = nc.sync.nop(nofuse=True, hint="drain_wait_collector")
    wait_clock.add_sem_waits(
        collector.ins, ScopedClock({None: tick_clock.global_clock})
    )
    si = collector.ins.sync_info
    waits = list(si.on_wait) if si and si.on_wait else []
    if len(waits) > _MAX_DRAIN_WAITS:
        si.on_wait = waits[:_MAX_DRAIN_WAITS]
        rest = waits[_MAX_DRAIN_WAITS:]
        while rest:
            chunk, rest = rest[:_MAX_DRAIN_WAITS], rest[_MAX_DRAIN_WAITS:]
            extra = nc.sync.nop(nofuse=True, hint="drain_wait_extra")
            if extra.ins.sync_info is None:
                extra.ins.sync_info = type(si)(on_wait=chunk, on_update=[])
            else:
                extra.ins.sync_info.on_wait = chunk

    nc.sync.drain()

    nc.all_engine_barrier()
    assert self.sems is not None
    popped = nc._tile_sem_poison_stack.pop()
    assert popped is self._sem_poison
    nc.clear_and_free_semaphores(list(self.sems.allocated().values()))
    nc.all_engine_barrier()


tile.TileContext._drain_and_barrier = _drain_and_barrier_split


def _split_excess_waits(nc):
    """Walrus in this env allows at most one sync wait per instruction;
    hoist extra waits onto same-engine NOPs placed just before."""
    import copy

    m = nc.m
    cnt = 0
    new_funcs = []
    for function in m.functions:
        new_function = copy.replace(function, blocks=[])
        new_function.set_allocations_from_list(function.allocations)
        for block in function.blocks:
            new_insts = []
            for inst in block.instructions:
                si = inst.sync_info
                waits = list(si.on_wait) if si and si.on_wait else []
                if len(waits) > 1:
                    for w in waits[:-1]:
                        nop = mybir.InstNoOp(name=f"I-swsplit-{cnt}",
                                             ins=[], outs=[])
                        cnt += 1
                        nop.engine = inst.engine
                        nop.sync_info = mybir.SyncInfo(on_wait=[w],
                                                       on_update=[])
                        new_insts.append(nop)
                    si.on_wait = [waits[-1]]
                new_insts.append(inst)
            new_function.blocks.append(
                copy.replace(block, instructions=new_insts))
        new_funcs.append(new_function)
    new_m = copy.replace(m, functions=[])
    for f in new_funcs:
        new_m.functions.append(f)
    nc.m = new_m
    return cnt


def build_nc(variant=None):
    variant = variant or os.environ.get("KVARIANT", "full")
    acc_split = os.environ.get("ACC_SPLIT", "dve")  # "dve" | "split"
    SCALE = float(np.float32(D) ** np.float32(-0.25))
    H4 = T // 4             # 512, quarter width
    A = mybir.AluOpType

    nc = bass.Bass()
    xT = nc.declare_dram_parameter("xT", [E, T], F16, isOutput=False)[:]
    Wq = nc.declare_dram_parameter("Wq", [128, EC * D], F16, isOutput=False)[:]
    Wk = nc.declare_dram_parameter("Wk", [128, EC * D], F16, isOutput=False)[:]
    Wv = nc.declare_dram_parameter("Wv", [128, EC * D], F16, isOutput=False)[:]
    bqc = nc.declare_dram_parameter("bqc", [D], F32, isOutput=False)[:]
    bkc = nc.declare_dram_parameter("bkc", [D], F32, isOutput=False)[:]
    bvc = nc.declare_dram_parameter("bvc", [D], F32, isOutput=False)[:]
    outT = nc.declare_dram_parameter("outT", [D, T], F32, isOutput=True)[:]

    with tile.TileContext(nc) as tc, \
         tc.tile_pool(name="consts", bufs=1) as consts, \
         tc.tile_pool(name="xpool", bufs=1) as xpool, \
         tc.tile_pool(name="persist", bufs=1) as persist, \
         tc.tile_pool(name="vtq", bufs=2) as vtqp, \
         tc.tile_pool(name="ppool", bufs=4) as ppool, \
         tc.tile_pool(name="fin", bufs=2) as finp, \
         tc.tile_pool(name="psO", bufs=2, space="PSUM") as psO, \
         tc.tile_pool(name="psS", bufs=2, space="PSUM") as psS, \
         tc.tile_pool(name="psP", bufs=2, space="PSUM") as psP:

        # ---- constants / weights (spread across both DGE queues) ----
        wq_s = consts.tile([128, EC, D], F16, tag="wq")
        wk_s = consts.tile([128, EC, D], F16, tag="wk")
        wv_s = consts.tile([128, EC, D], F16, tag="wv")
        wk_r = Wk.rearrange("p (c d) -> p c d", d=D)
        bq_s = consts.tile([128, 1], F32, tag="bq")
        bk_s = consts.tile([128, 1], F32, tag="bk")
        bv_s = consts.tile([128, 1], F32, tag="bv")
        ident = consts.tile([128, 128], F32, tag="ident")
        onec = consts.tile([128, 1], BF16, tag="onec")
        oner = consts.tile([1, 128], F16, tag="oner")

        # ---- x: 8 full [128, T] rows on 2 HW DGE queues (fewer, bigger
        # DMAs -- per-queue slots cost ~0.6us each regardless of size);
        # tiny consts are built on-device / sent via the gpsimd SW queue ----
        xh = [xpool.tile([128, T], F16, tag=f"x{e}", name=f"xh{e}")
              for e in range(EC)]

        def xdma(eng, e):
            eng.dma_start(out=xh[e], in_=xT[e * 128:(e + 1) * 128, :])

        nc.sync.dma_start(out=wk_s[:, 0:2, :], in_=wk_r[:, 0:2, :])
        xdma(nc.sync, 0)
        xdma(nc.sync, 2)
        nc.sync.dma_start(out=wk_s[:, 2:EC, :], in_=wk_r[:, 2:EC, :])
        xdma(nc.sync, 4)
        xdma(nc.sync, 6)
        xdma(nc.scalar, 1)
        xdma(nc.scalar, 3)
        nc.scalar.dma_start(out=wv_s, in_=Wv.rearrange("p (c d) -> p c d",
                                                       d=D))
        xdma(nc.scalar, 5)
        xdma(nc.scalar, 7)
        nc.scalar.dma_start(out=wq_s,
                            in_=Wq.rearrange("p (c d) -> p c d", d=D))
        for b_s, b_d in ((bq_s, bqc), (bk_s, bkc), (bv_s, bvc)):
            nc.gpsimd.dma_start(out=b_s, in_=b_d.unsqueeze(1))
        make_identity(nc, ident)
        nc.gpsimd.memset(onec, 1.0)
        nc.gpsimd.memset(oner, 1.0)

        # ---- HAM warm-up: junk matmuls on the first-arrived weight chunk
        # fill the DMA lead-in so the PE clock gate opens before real work
        n_dummy = int(os.environ.get("DUMMY_MM", "12"))
        if n_dummy:
            dmy_ps = psP.tile([128, 256], F32, tag="pj", name="dummy_ps")
            wk01 = wk_s.rearrange("p c d -> p (c d)")[:, 0:2 * D]
            for _ in range(n_dummy):
                nc.tensor.matmul(dmy_ps, wk_s[:, 0, :], wk01,
                                 start=True, stop=True)

        def xq(e, h):
            # [128, 512] view of quarter h of E-chunk e
            return xh[e][:, h * H4:(h + 1) * H4]

        kT_s = persist.tile([128, T], F16, tag="kT")
        qT_s = persist.tile([128, T], F16, tag="qT")
        V_s = persist.tile([128, NKB, D], BF16, tag="V")
        acc = [persist.tile([128, SPAN], BF16, tag=f"acc{s}", name=f"acc{s}")
               for s in range(NSPAN)]

        ot_ps = [None] * NSPAN      # span -> persistent PSUM bank
        done_in_span = [0] * NSPAN  # kb-pairs accumulated so far

        def proj_quarter(h):
            hsl = slice(h * H4, (h + 1) * H4)
            k_ps = psP.tile([128, H4], F32, tag="pj", name=f"k_ps{h}")
            for e in range(EC):
                nc.tensor.matmul(k_ps, wk_s[:, e, :], xq(e, h),
                                 start=(e == 0), stop=(e == EC - 1))
            nc.vector.tensor_scalar(out=kT_s[:, hsl], in0=k_ps,
                                    scalar1=bk_s, scalar2=SCALE,
                                    op0=A.add, op1=A.mult)
            v_ps = psP.tile([128, H4], F32, tag="pj", name=f"v_ps{h}")
            for e in range(EC):
                nc.tensor.matmul(v_ps, wv_s[:, e, :], xq(e, h),
                                 start=(e == 0), stop=(e == EC - 1))
            vtq = vtqp.tile([128, H4], F32, tag="vtq", name=f"vtq{h}")
            nc.vector.tensor_scalar(out=vtq, in0=v_ps,
                                    scalar1=bv_s, scalar2=None, op0=A.add)
            q_ps = psP.tile([128, H4], F32, tag="pj", name=f"q_ps{h}")
            for e in range(EC):
                nc.tensor.matmul(q_ps, wq_s[:, e, :], xq(e, h),
                                 start=(e == 0), stop=(e == EC - 1))
            nc.vector.tensor_scalar(out=qT_s[:, hsl], in0=q_ps,
                                    scalar1=bq_s, scalar2=SCALE,
                                    op0=A.add, op1=A.mult)
            # V natural layout [k, D] for the 4 new key blocks
            for j in range(4):
                kb = 4 * h + j
                vt_ps = psP.tile([128, 128], F32, tag="pj", name=f"vt{kb}")
                nc.tensor.transpose(vt_ps, vtq[:, j * 128:(j + 1) * 128],
                                    ident)
                nc.vector.tensor_copy(out=V_s[:, kb, :], in_=vt_ps)

        def unit(p, s):
            # one kb-pair (blocks 2p, 2p+1) against span s
            ssl = slice(s * SPAN, (s + 1) * SPAN)
            if ot_ps[s] is None:
                ot_ps[s] = psO.tile([128, SPAN], F32, tag="ot",
                                    name=f"ot{s}")
            st2 = psS.tile([128, 2 * SPAN], F32, tag="st", name=f"st{p}_{s}")
            for i in range(2):
                kb = 2 * p + i
                nc.tensor.matmul(st2[:, i * SPAN:(i + 1) * SPAN],
                                 kT_s[:, kb * 128:(kb + 1) * 128],
                                 qT_s[:, ssl], start=True, stop=True)
            p2 = ppool.tile([128, 2 * SPAN], BF16, tag="p2", name=f"p{p}_{s}")
            nc.scalar.activation(out=p2, in_=st2,
                                 func=mybir.ActivationFunctionType.Exp)
            first = done_in_span[s] == 0
            for i in range(2):
                kb = 2 * p + i
                nc.tensor.matmul(ot_ps[s], V_s[:, kb, :],
                                 p2[:, i * SPAN:(i + 1) * SPAN],
                                 start=(first and i == 0),
                                 stop=(done_in_span[s] == NPAIR - 1
                                       and i == 1))
            # denominator partial sums (fp16, values <= ~2.4e3 << 65504)
            if first:
                nc.vector.tensor_tensor(out=acc[s], in0=p2[:, 0:SPAN],
                                        in1=p2[:, SPAN:], op=A.add)
            else:
                eng = nc.vector
                if acc_split == "split" and (p + s) % 2 == 1:
                    eng = nc.gpsimd
                eng.tensor_tensor(out=acc[s], in0=acc[s], in1=p2[:, 0:SPAN],
                                  op=A.add)
                eng.tensor_tensor(out=acc[s], in0=acc[s], in1=p2[:, SPAN:],
                                  op=A.add)
            done_in_span[s] += 1

        def finalize(s):
            ssl = slice(s * SPAN, (s + 1) * SPAN)
            lr_ps = psP.tile([1, SPAN], F32, tag="pj", name=f"lr{s}")
            nc.tensor.matmul(lr_ps, onec, acc[s], start=True, stop=True)
            # 1/l = exp(-ln(l)) on ACT -- walrus here rejects the custom-DVE
            # fast-reciprocal, and plain DVE reciprocal is ~8 cyc/elem.
            lg = finp.tile([1, SPAN], F32, tag="lg", name=f"lg{s}")
            nc.scalar.activation(out=lg, in_=lr_ps,
                                 func=mybir.ActivationFunctionType.Ln)
            rl16 = finp.tile([1, SPAN], F16, tag="rl16", name=f"rl16{s}")
            nc.scalar.activation(out=rl16, in_=lg, scale=-1.0,
                                 func=mybir.ActivationFunctionType.Exp)
            rlb_ps = psP.tile([128, SPAN], F32, tag="pj", name=f"rlb{s}")
            nc.tensor.matmul(rlb_ps, oner, rl16, start=True, stop=True)
            rlb = finp.tile([128, SPAN], F32, tag="rlb", name=f"rlbs{s}")
            nc.scalar.copy(out=rlb, in_=rlb_ps)
            outsp = finp.tile([128, SPAN], F32, tag="out", name=f"out{s}")
            half = SPAN // 2
            for i, eng in enumerate((nc.sync, nc.scalar)):
                hs = slice(i * half, (i + 1) * half)
                nc.vector.tensor_tensor(out=outsp[:, hs],
                                        in0=ot_ps[s][:, hs],
                                        in1=rlb[:, hs], op=A.mult)
                eng.dma_start(
                    out=outT[:, s * SPAN + i * half:s * SPAN + (i + 1) * half],
                    in_=outsp[:, hs])

        # ---- pipelined schedule ----
        # sweep A: spans 0,1 interleaved with projections as kT/qT arrive
        proj_quarter(0)
        unit(0, 0); unit(1, 0)
        proj_quarter(1)
        unit(2, 0); unit(3, 0)
        unit(0, 1); unit(1, 1); unit(2, 1); unit(3, 1)
        proj_quarter(2)
        unit(4, 0); unit(4, 1); unit(5, 0); unit(5, 1)
        proj_quarter(3)
        unit(6, 0); unit(6, 1); unit(7, 0)
        finalize(0)
        unit(7, 1)
        finalize(1)
        # sweep B: spans 2,3 (kT/qT/V all resident now)
        for p in range(NPAIR - 1):
            unit(p, 2)
            unit(p, 3)
        unit(NPAIR - 1, 2)
        finalize(2)
        unit(NPAIR - 1, 3)
        finalize(3)

    return nc


_CACHED = {}


def _get_nc(key="v2"):
    if key not in _CACHED:
        nc = build_nc()
        _split_excess_waits(nc)
        _CACHED[key] = nc
    return _CACHED[key]


def _make_in_maps(x, Wq, bq, Wk, bk, Wv, bv):
    def rnd16(a):
        return np.ascontiguousarray(np.asarray(a, np.float32), np.float16)

    xTm = rnd16(np.transpose(np.asarray(x, np.float32), (0, 2, 1)))

    def warr(w):
        w = np.asarray(w, np.float32).reshape(EC, 128, D)
        return rnd16(w.transpose(1, 0, 2).reshape(128, EC * D))

    Wq, Wk, Wv = warr(Wq), warr(Wk), warr(Wv)
    bqc = np.ascontiguousarray(np.asarray(bq, np.float32))
    bkc = np.ascontiguousarray(np.asarray(bk, np.float32))
    bvc = np.ascontiguousarray(np.asarray(bv, np.float32))
    return [
        {"xT": np.ascontiguousarray(xTm[b]), "Wq": Wq, "Wk": Wk, "Wv": Wv,
         "bqc": bqc, "bkc": bkc, "bvc": bvc}
        for b in range(B)
    ]


def kernel(x, Wq, bq, Wk, bk, Wv, bv, _trace=False, _mm_dt=None):
    from concourse.bass_utils import run_bass_kernel_spmd

    nc = _get_nc()
    in_maps = _make_in_maps(x, Wq, bq, Wk, bk, Wv, bv)
    res = run_bass_kernel_spmd(nc, in_maps, core_ids=list(range(B)),
                               trace=_trace)
    out = np.stack([np.ascontiguousarray(res.results[b]["outT"].T)
                    for b in range(B)])
    kernel._last_result = res
    return out
